# revision 5
# baseline (speedup 1.0000x reference)
"""Trainium2 Bass kernel for an image-captioning decoder:
embedding lookup -> 21-step LSTM (B=64, H=512) -> vocab projection
[1280,512]@[512,32000] -> softmax over V=32000.

Sharding: vocab-parallel across 8 cores (4000 vocab cols each); the LSTM is
computed redundantly on every core (its cost is batch-independent weight
streaming).  Each core emits exp(logits)/local_sum plus its local row sums;
the host rescales rows by local_sum/global_sum while assembling the full
output (exact softmax, no cross-core collective needed on device).

Layout/algebra tricks:
  - gates ordered [g,f,i,o]; sigmoid(x) = 0.5*(1+tanh(x/2)) so every ACT
    function used (tanh/exp) lives in one table set (no table reloads).
  - h is kept doubled (h~ = 2h) so the 0.5 factors fold into w_hh and fc_w
    on the host.
  - PE matmuls run as float32r (1 cyc/row); the vocab projection uses bf16
    weights/activations.
  - vocab-projection m-tiles (2 timesteps each) are interleaved into the
    recurrence as PE gap-filler; exp/normalize/store pipeline behind them.
"""

import numpy as np
import ml_dtypes

import concourse.bass as bass
import concourse.mybir as mybir
import concourse.tile as tile
from concourse import bacc
from concourse.bass_utils import run_bass_kernel_spmd

# problem shapes (hardcoded per contract)
E = 512
H = 512
V = 32000
B = 64
T = 20
S = T + 1          # lstm steps (features + T embeddings)
G4 = 4 * H         # 2048 gate pre-activations
NCORES = 8
VS = V // NCORES   # 4000 vocab cols per core
NM_X = 11          # x row-chunks of 128 (2 steps each; padded 1344->1408)
RXP = NM_X * 128
NM_L = 10          # logits row-chunks of 128 (2 timesteps each)
NCH = 8            # vocab chunks per core (4000 = 8 * 500)
CH = VS // NCH     # 500

F32 = mybir.dt.float32
F32R = mybir.dt.float32r
BF16 = mybir.dt.bfloat16
I32 = mybir.dt.int32
AF = mybir.ActivationFunctionType
ALU = mybir.AluOpType
AX = mybir.AxisListType


def build_nc(n_cores=NCORES):
    nc = bacc.Bacc(
        "TRN2",
        target_bir_lowering=False,
        debug=False,
        enable_asserts=True,
        num_devices=n_cores,
    )

    # ---- I/O ----
    feat = nc.dram_tensor("features", [B, E], F32, kind="ExternalInput").ap()
    emb = nc.dram_tensor("emb_table", [V, E], F32, kind="ExternalInput").ap()
    idxd = nc.dram_tensor("idx_all", [128, NM_X], I32, kind="ExternalInput").ap()
    wihd = nc.dram_tensor("w_ihT", [4, 128, G4], F32R, kind="ExternalInput").ap()
    whhd = nc.dram_tensor("w_hhT", [4, 128, G4], F32R, kind="ExternalInput").ap()
    biasd = nc.dram_tensor("bias", [128, G4], F32, kind="ExternalInput").ap()
    fcwd = nc.dram_tensor("fc_wT", [4, 128, VS], BF16, kind="ExternalInput").ap()
    fcbd = nc.dram_tensor("fc_b", [8, VS], BF16, kind="ExternalInput").ap()
    idend = nc.dram_tensor("iden", [128, 128], F32, kind="ExternalInput").ap()
    idenrd = nc.dram_tensor("iden_r", [128, 128], F32R, kind="ExternalInput").ap()
    onesd = nc.dram_tensor("ones", [8, 128], BF16, kind="ExternalInput").ap()
    outd = nc.dram_tensor("out", [B, T, VS], F32, kind="ExternalOutput").ap()
    sumsd = nc.dram_tensor("sums_out", [128, NM_L], F32, kind="ExternalOutput").ap()
    out_tbv = outd.rearrange("b t v -> t b v")  # [20, 64, VS]

    with tile.TileContext(nc) as tc:
        with (
            tc.tile_pool(name="consts", bufs=1) as constP,
            tc.tile_pool(name="state", bufs=1) as stateP,
            tc.tile_pool(name="hroll", bufs=2) as hrollP,
            tc.tile_pool(name="weights", bufs=1) as wP,
            tc.tile_pool(name="xwin", bufs=2) as xwP,
            tc.tile_pool(name="xgwin", bufs=2) as xgP,
            tc.tile_pool(name="work1", bufs=2) as w1P,   # act outputs
            tc.tile_pool(name="work2", bufs=1) as w2P,   # chain temps
            tc.tile_pool(name="expP", bufs=1) as exP,
            tc.tile_pool(name="accP", bufs=2) as accP,
            tc.tile_pool(name="psA", bufs=1, space="PSUM") as psA,
            tc.tile_pool(name="psB", bufs=2, space="PSUM") as psB,
            tc.tile_pool(name="psG", bufs=2, space="PSUM") as psG,
            tc.tile_pool(name="psT", bufs=1, space="PSUM") as psT,
            tc.tile_pool(name="psC", bufs=2, space="PSUM") as psC,
        ):
            # ---- constants / persistent state ----
            id_sb = constP.tile([128, 128], F32)
            nc.sync.dma_start(id_sb[:], idend)
            idr_sb = constP.tile([128, 128], F32R)
            nc.sync.dma_start(idr_sb[:], idenrd)
            ones_sb = constP.tile([8, 128], BF16)
            nc.sync.dma_start(ones_sb[:], onesd)
            idx_sb = constP.tile([128, NM_X], I32)
            nc.sync.dma_start(idx_sb[:], idxd)
            fcb_sb = constP.tile([8, VS], BF16)
            nc.sync.dma_start(fcb_sb[:], fcbd)

            hsT = stateP.tile([128, 4, T * B], BF16)  # 2*h_1..2*h_20 transposed
            c_sb = stateP.tile([B, H], F32)           # cell state
            sums = stateP.tile([128, NM_L], F32)      # local exp row-sums

            bias_sb = wP.tile([128, G4], F32)
            nc.sync.dma_start(bias_sb[:], biasd)
            wih = wP.tile([128, 4, G4], F32R)
            whh = wP.tile([128, 4, G4], F32R)
            fcw = wP.tile([128, 4, VS], BF16)
            for k in range(4):
                nc.sync.dma_start(wih[:, k, :], wihd[k])
                nc.sync.dma_start(whh[:, k, :], whhd[k])
                nc.sync.dma_start(fcw[:, k, :], fcwd[k])

            # ---- phase A: gather + transpose + xgates (windowed) ----
            # emitted in m-order; tile slot pacing interleaves it with B
            xg_tiles = []
            for m in range(NM_X):
                x_raw = xwP.tile([128, E], F32, tag="xraw")
                nc.gpsimd.indirect_dma_start(
                    out=x_raw[:],
                    out_offset=None,
                    in_=emb,
                    in_offset=bass.IndirectOffsetOnAxis(
                        ap=idx_sb[:, m : m + 1], axis=0
                    ),
                )
                if m == 0:
                    # rows 0:64 are t=0 -> image features
                    nc.sync.dma_start(x_raw[0:B, :], feat)
                xt_ps = psA.tile([128, 512], F32, tag="xtps")
                for k in range(4):
                    nc.tensor.transpose(
                        xt_ps[:, k * 128 : (k + 1) * 128],
                        x_raw[:, k * 128 : (k + 1) * 128],
                        id_sb[:],
                    )
                xt = xwP.tile([128, 4, 128], F32R, tag="xt")
                nc.vector.tensor_copy(
                    xt[:], xt_ps[:].rearrange("p (k c) -> p k c", k=4)
                )
                xg = xgP.tile([128, G4], F32R, tag="xg")
                for n in range(4):
                    ps = psB.tile([128, 512], F32, tag="xgps")
                    for k in range(4):
                        nc.tensor.matmul(
                            ps[:],
                            xt[:, k, :],
                            wih[:, k, n * 512 : (n + 1) * 512],
                            start=(k == 0),
                            stop=(k == 3),
                        )
                    nc.vector.tensor_tensor(
                        out=xg[:, n * 512 : (n + 1) * 512],
                        in0=ps[:],
                        in1=bias_sb[:, n * 512 : (n + 1) * 512],
                        op=ALU.add,
                    )
                xg_tiles.append(xg)

            # ---- phase B: recurrence + interleaved vocab projection ----
            hT_prev = None
            for s in range(S):
                xg = xg_tiles[s // 2]
                half = idr_sb[:, 64 * (s % 2) : 64 * (s % 2) + 64]
                # gate chunk order: 0=g, 1=f, 2=i, 3=o
                acts = {}
                for n in range(4):
                    if s == 0 and n == 1:
                        continue  # f unused when c=0
                    sl = slice(n * 512, (n + 1) * 512)
                    pg = psG.tile([B, 512], F32, tag="psg")
                    nc.tensor.matmul(
                        pg[:], half, xg[:, sl],
                        start=True, stop=(s == 0),
                    )
                    if s > 0:
                        for k in range(4):
                            nc.tensor.matmul(
                                pg[:], hT_prev[:, k, :], whh[:, k, sl],
                                start=False, stop=(k == 3),
                            )
                    name = "gfio"[n]
                    t = w1P.tile([B, 512], F32, tag=name)
                    # g: tanh(x); f,i,o: tanh(x/2) (sigmoid via tanh)
                    nc.scalar.activation(
                        t[:], pg[:], AF.Tanh, scale=(1.0 if n == 0 else 0.5)
                    )
                    acts[name] = t
                g_t, ip = acts["g"], acts["i"]
                # c' = 0.5*((c + f'c) + (g + i'g)) ; at s=0: c' = 0.5*(g + i'g)
                v = w2P.tile([B, H], F32, tag="v")
                nc.vector.tensor_tensor(out=v[:], in0=ip[:], in1=g_t[:], op=ALU.mult)
                z = w2P.tile([B, H], F32, tag="z")
                nc.vector.tensor_tensor(out=z[:], in0=v[:], in1=g_t[:], op=ALU.add)
                if s > 0:
                    fp = acts["f"]
                    u = w2P.tile([B, H], F32, tag="u")
                    nc.vector.tensor_tensor(out=u[:], in0=fp[:], in1=c_sb[:], op=ALU.mult)
                    w_ = w2P.tile([B, H], F32, tag="w")
                    nc.vector.tensor_tensor(out=w_[:], in0=u[:], in1=c_sb[:], op=ALU.add)
                    tmp = w2P.tile([B, H], F32, tag="tmp")
                    nc.vector.tensor_tensor(out=tmp[:], in0=w_[:], in1=z[:], op=ALU.add)
                else:
                    tmp = z
                nc.vector.tensor_scalar_mul(c_sb[:], tmp[:], 0.5)
                tc_t = w1P.tile([B, H], F32, tag="tc")
                nc.scalar.activation(tc_t[:], c_sb[:], AF.Tanh)
                # h~ = 2h = tanh(c) + o' * tanh(c)
                d = w2P.tile([B, H], F32, tag="d")
                nc.vector.tensor_tensor(out=d[:], in0=acts["o"][:], in1=tc_t[:], op=ALU.mult)
                h_sb = w1P.tile([B, H], F32, tag="h")
                nc.vector.tensor_tensor(out=h_sb[:], in0=tc_t[:], in1=d[:], op=ALU.add)
                # transpose h~ -> [128, 4, 64]
                pst = psT.tile([128, 4 * B], F32, tag="pst")
                for k in range(4):
                    nc.tensor.transpose(
                        pst[:, k * B : (k + 1) * B],
                        h_sb[:, k * 128 : (k + 1) * 128],
                        id_sb[0:B, 0:B],
                    )
                hT_prev = hrollP.tile([128, 4, B], F32R, tag="hT")
                pst4 = pst[:].rearrange("p (k b) -> p k b", k=4)
                nc.vector.tensor_copy(hT_prev[:], pst4)
                if s >= 1:
                    nc.vector.tensor_copy(hsT[:, :, (s - 1) * B : s * B], pst4)

                # ---- interleaved vocab projection for ready m-tile ----
                if s >= 2 and s % 2 == 0:
                    m = (s - 2) // 2
                    _vocab_mtile(nc, m, hsT, fcw, fcb_sb, ones_sb, sums,
                                 exP, accP, psC, out_tbv)
            # last m-tile (rows from steps 19,20)
            _vocab_mtile(nc, NM_L - 1, hsT, fcw, fcb_sb, ones_sb, sums,
                         exP, accP, psC, out_tbv)
            nc.sync.dma_start(sumsd, sums[:])

    nc.compile()
    return nc


def _vocab_mtile(nc, m, hsT, fcw, fcb_sb, ones_sb, sums, exP, accP, psC, out_tbv):
    """logits -> exp -> local row-sum -> normalize -> store, for one
    128-row m-tile (timesteps 2m, 2m+1)"""
    acc = accP.tile([128, NCH], F32, tag="acc")
    ex = exP.tile([128, VS], F32, tag="ex")
    for j in range(NCH):
        sl = slice(j * CH, (j + 1) * CH)
        ps = psC.tile([128, CH], F32, tag="lg")
        # fc_b via K=8 matmul of ones/8 (resets psum), then 4 k-tiles
        nc.tensor.matmul(
            ps[:], ones_sb[:, :], fcb_sb[:, sl],
            start=True, stop=False,
        )
        for k in range(4):
            nc.tensor.matmul(
                ps[:],
                hsT[:, k, m * 128 : (m + 1) * 128],
                fcw[:, k, sl],
                start=False, stop=(k == 3),
            )
        nc.scalar.activation(
            ex[:, sl], ps[:], AF.Exp, accum_out=acc[:, j : j + 1]
        )
    nc.vector.reduce_sum(sums[:, m : m + 1], acc[:], axis=AX.X)
    inv = accP.tile([128, 1], F32, tag="inv")
    nc.vector.reciprocal(inv[:], sums[:, m : m + 1])
    # normalize in place (alternate DVE / GpSimd to balance engines)
    eng = nc.vector if m % 2 == 0 else nc.gpsimd
    eng.tensor_scalar_mul(ex[:], ex[:], inv[:])
    nc.sync.dma_start(out_tbv[2 * m, :, :], ex[0:B, :])
    nc.sync.dma_start(out_tbv[2 * m + 1, :, :], ex[B:128, :])


def prep_inputs(features, captions, lenghts, emb_table, w_ih, w_hh,
                b_ih, b_hh, fc_w, fc_b):
    """host-side prep: dtype casts, weight transposes, gate reorder, shards"""
    f32 = np.float32
    bf16 = ml_dtypes.bfloat16
    features = np.ascontiguousarray(np.asarray(features), dtype=f32)
    captions = np.asarray(captions).astype(np.int32)
    emb_table = np.ascontiguousarray(np.asarray(emb_table), dtype=f32)
    w_ih = np.asarray(w_ih, dtype=f32)
    w_hh = np.asarray(w_hh, dtype=f32)
    bias = (np.asarray(b_ih, dtype=f32) + np.asarray(b_hh, dtype=f32))
    fc_w = np.asarray(fc_w, dtype=f32)
    fc_b = np.asarray(fc_b, dtype=f32)

    # gate reorder: torch order i,f,g,o -> our layout g,f,i,o
    perm = np.r_[1024:1536, 512:1024, 0:512, 1536:2048]
    w_ihT = np.ascontiguousarray(w_ih[perm].T).reshape(4, 128, G4)
    # h is stored doubled -> halve w_hh
    w_hhT = np.ascontiguousarray(0.5 * w_hh[perm].T).reshape(4, 128, G4)
    bias = np.ascontiguousarray(
        np.broadcast_to(bias[perm][None, :], (128, G4)))

    # t-major row index table for the embedding gather (padded to 1408)
    idx = np.zeros(RXP, dtype=np.int32)
    idx[B : B * S] = captions.T.ravel()
    idx_all = np.ascontiguousarray(idx.reshape(NM_X, 128).T)  # [128, NM_X]

    iden = np.eye(128, dtype=f32)
    ones8 = np.ones((8, 128), dtype=f32).astype(bf16)

    common = dict(
        features=features,
        emb_table=emb_table,
        idx_all=idx_all,
        w_ihT=w_ihT,
        w_hhT=w_hhT,
        bias=bias,
        iden=iden,
        iden_r=iden,
        ones=ones8,
    )
    in_maps = []
    for c in range(NCORES):
        sl = slice(c * VS, (c + 1) * VS)
        m = dict(common)
        # h~ = 2h -> halve fc_w; bf16 for the vocab projection
        m["fc_wT"] = np.ascontiguousarray(
            0.5 * fc_w[sl].T).reshape(4, 128, VS).astype(bf16)
        m["fc_b"] = np.ascontiguousarray(
            np.broadcast_to(fc_b[sl][None, :] / 8.0, (8, VS))).astype(bf16)
        in_maps.append(m)
    return in_maps


def assemble(results):
    """combine per-core (out, sums_out) into the full softmax output"""
    # local sums per core: [128, NM_L]; row r = m*128+p maps to (t=r//64, b=r%64)
    loc = []
    for r in results:
        s = np.asarray(r["sums_out"], dtype=np.float64)  # [128, 10]
        loc.append(s.T.reshape(T, B).T)  # [64, 20]
    loc = np.stack(loc)                  # [8, 64, 20]
    tot = loc.sum(axis=0)                # [64, 20]
    parts = []
    for c, r in enumerate(results):
        scale = (loc[c] / tot).astype(np.float32)  # [64, 20]
        parts.append(r["out"] * scale[:, :, None])
    return np.concatenate(parts, axis=2)


_NC_CACHE = {}


def _get_nc(n_cores=NCORES):
    if n_cores not in _NC_CACHE:
        _NC_CACHE[n_cores] = build_nc(n_cores)
    return _NC_CACHE[n_cores]


def kernel(features, captions, lenghts, emb_table, w_ih, w_hh,
           b_ih, b_hh, fc_w, fc_b, _trace=False, _results=None):
    in_maps = prep_inputs(features, captions, lenghts, emb_table,
                          w_ih, w_hh, b_ih, b_hh, fc_w, fc_b)
    nc = _get_nc(NCORES)
    res = run_bass_kernel_spmd(
        nc, in_maps, core_ids=list(range(NCORES)), trace=_trace
    )
    if _results is not None:
        _results.append(res)
    return assemble(res.results)
